# revision 13
# baseline (speedup 1.0000x reference)
"""Trainium2 Bass kernel for nn_Aligner (B=16, C=D=512, N1=N2=2048).

  scores[b,n2,n1] = sum_c RI_2[b,c,n2] * RI_1[b,c,n1]
  attention_map   = softmax(scores, axis=1)          # over n2
  RE_embed[b,d,n1]= sum_n2 RE_2[b,d,n2] * attention_map[b,n2,n1]
  returns (RE_embed, attention_map)

Sharding: data-parallel over batch, 2 batches per core on 8 cores.

Per-core plan (per batch, per n1-chunk i of 128):
  MM1:  ST_i[n1,n2] = RI_1_chunk.T @ RI_2  (scores transposed, so softmax
        reduces along the free axis).  PSUM [128, 4x512] (4 banks).
  softmax: DVE reduce_max per 512-subtile -> combine -> ACT exp with
        bias=-max and accum_out partial sums -> DVE reciprocal ->
        DVE tensor_scalar mul (normalize).  AT_i [128, 2048] in SBUF.
  PE transposes: 16x [128,128] blocks AT_i -> A natural layout blocks
        (n2 partitions), via PSUM, ACT-copied to SBUF.
  MM2:  ET_i[n1, d] = sum_j A_block_j.T @ RE_2T_j, PSUM accumulate over
        16 n2-chunks.  Stored transposed; host transposes RE_embed back.
  DMA out: A natural blocks -> attention_map[b, :, i*128:(i+1)*128],
        ET_i -> et[b, i*128:(i+1)*128, :].
"""

import numpy as np

# Matmul precision modes: "f32" (exact, 4 cyc/row), "f32r" (1 cyc/row,
# reduced-precision fp32 streaming), "f16x3" (3x fp16 matmuls w/ hi+lo
# split, ~fp32 accuracy at 3 cyc/row).
MM1_MODE = "f32"
MM2_MODE = "f32"

P = 128
BP = 2            # batches per core
C = 512
CCH = C // P      # 4 contraction chunks
N1 = 2048
N1CH = N1 // P    # 16 n1 chunks
N2 = 2048
N2SUB = N2 // 512  # 4 n2 subtiles of 512 for MM1
NJ = N2 // P      # 16 n2 chunks of 128 for transposes / MM2
D = 512

_CACHE = {}


def _build(reps=1, timing=False):
    from contextlib import ExitStack

    import concourse.bacc as bacc
    import concourse.mybir as mybir
    import concourse.tile as tile
    from concourse.masks import make_identity

    dt = mybir.dt
    f32 = dt.float32
    f32r = dt.float32r
    f16 = dt.float16

    nc = bacc.Bacc("TRN2", target_bir_lowering=False, debug=False, num_devices=8)

    # DRAM I/O (per core shard: 2 batches).  In timing mode all large
    # tensors are Internal (device scratch, no host transfer) so wall
    # time ~= dispatch + HW exec; data is garbage but timing-equivalent.
    kin = "Internal" if timing else "ExternalInput"
    kout = "Internal" if timing else "ExternalOutput"
    if MM1_MODE == "f16x3":
        ri1_hi = nc.dram_tensor("ri1_hi", [BP, C, N1], f16, kind=kin).ap()
        ri1_lo = nc.dram_tensor("ri1_lo", [BP, C, N1], f16, kind=kin).ap()
        ri2_hi = nc.dram_tensor("ri2_hi", [BP, C, N2], f16, kind=kin).ap()
        ri2_lo = nc.dram_tensor("ri2_lo", [BP, C, N2], f16, kind=kin).ap()
    else:
        ri1_d = nc.dram_tensor("ri1", [BP, C, N1], f32, kind=kin).ap()
        ri2_d = nc.dram_tensor("ri2", [BP, C, N2], f32, kind=kin).ap()
    re2t_d = nc.dram_tensor("re2t", [BP, N2, D], f32, kind=kin).ap()
    attn_d = nc.dram_tensor("attn", [BP, N2, N1], f32, kind=kout).ap()
    et_d = nc.dram_tensor("et", [BP, N1, D], f32, kind=kout).ap()
    dummy_d = None
    if timing:
        dummy_d = nc.dram_tensor("tout", [P, 1], f32, kind="ExternalOutput").ap()

    with ExitStack() as ctx:
        tc = ctx.enter_context(tile.TileContext(nc))
        const = ctx.enter_context(tc.tile_pool(name="const", bufs=1))
        ident = const.tile([P, P], f32)
        make_identity(nc, ident)

        ri_pool = ctx.enter_context(tc.tile_pool(name="ri", bufs=2))
        re_pool = ctx.enter_context(tc.tile_pool(name="re", bufs=1))
        at_pool = ctx.enter_context(tc.tile_pool(name="at", bufs=2))
        anat_pool = ctx.enter_context(tc.tile_pool(name="anat", bufs=2))
        et_pool = ctx.enter_context(tc.tile_pool(name="etsb", bufs=2))
        stat_pool = ctx.enter_context(tc.tile_pool(name="stat", bufs=3))
        st_pool = ctx.enter_context(tc.tile_pool(name="st", bufs=1, space="PSUM"))
        tp_pool = ctx.enter_context(tc.tile_pool(name="tp", bufs=2, space="PSUM"))
        etp_pool = ctx.enter_context(tc.tile_pool(name="etp", bufs=2, space="PSUM"))

        def mm1_cast(ap):
            return ap.bitcast(f32r) if MM1_MODE == "f32r" else ap

        def mm2_cast(ap):
            return ap.bitcast(f32r) if MM2_MODE == "f32r" else ap

        def emit_tail(at, re2t, b, i):
            """Transposes + MM2 + output DMAs for chunk (b, i)."""
            anat = anat_pool.tile([P, NJ, P], f32, tag="anat")
            anat_flat = anat.rearrange("p j n -> p (j n)")
            for g in range(4):
                tp = tp_pool.tile([P, 4 * P], f32, tag="tp")
                for k in range(4):
                    jj = g * 4 + k
                    nc.tensor.transpose(
                        tp[:, k * P:(k + 1) * P],
                        at[:, jj * P:(jj + 1) * P],
                        ident,
                    )
                nc.scalar.copy(anat_flat[:, g * 512:(g + 1) * 512], tp[:])
            # attention_map natural-layout store
            nc.sync.dma_start(
                attn_d[b].rearrange("(j p) n -> p j n", p=P)[:, :, i * P:(i + 1) * P],
                anat,
            )
            # MM2: ET_i[n1,d] accumulate over n2 chunks
            etp = etp_pool.tile([P, D], f32, tag="etp")
            for jj in range(NJ):
                nc.tensor.matmul(
                    etp[:],
                    mm2_cast(anat[:, jj, :]),
                    mm2_cast(re2t[:, jj, :]),
                    start=(jj == 0),
                    stop=(jj == NJ - 1),
                )
            et_sb = et_pool.tile([P, D], f32, tag="et_sb")
            nc.vector.tensor_copy(et_sb[:], etp[:])
            nc.sync.dma_start(et_d[b, i * P:(i + 1) * P, :], et_sb[:])

        def emit_workload():
         pending = None
         for b in range(BP):
            if MM1_MODE == "f16x3":
                r1h = ri_pool.tile([P, CCH, N1], f16, tag="r1h")
                r1l = ri_pool.tile([P, CCH, N1], f16, tag="r1l")
                r2h = ri_pool.tile([P, CCH, N2], f16, tag="r2h")
                r2l = ri_pool.tile([P, CCH, N2], f16, tag="r2l")
                nc.sync.dma_start(r1h[:], ri1_hi[b].rearrange("(c p) n -> p c n", p=P))
                nc.sync.dma_start(r1l[:], ri1_lo[b].rearrange("(c p) n -> p c n", p=P))
                nc.sync.dma_start(r2h[:], ri2_hi[b].rearrange("(c p) n -> p c n", p=P))
                nc.sync.dma_start(r2l[:], ri2_lo[b].rearrange("(c p) n -> p c n", p=P))
                mm1_pairs = [(r1h, r2h), (r1h, r2l), (r1l, r2h)]
            else:
                ri1 = ri_pool.tile([P, CCH, N1], f32, tag="ri1")
                ri2 = ri_pool.tile([P, CCH, N2], f32, tag="ri2")
                nc.sync.dma_start(ri1[:], ri1_d[b].rearrange("(c p) n -> p c n", p=P))
                nc.sync.dma_start(ri2[:], ri2_d[b].rearrange("(c p) n -> p c n", p=P))
                mm1_pairs = [(ri1, ri2)]
            re2t = re_pool.tile([P, NJ, D], f32, tag="re2t")
            nc.sync.dma_start(re2t[:], re2t_d[b].rearrange("(j p) d -> p j d", p=P))

            for i in range(N1CH):
                # ---- MM1: ST_i [128 (n1), 2048 (n2)] in PSUM ----
                st = st_pool.tile([P, N2SUB, 512], f32, tag="st")
                pmax = stat_pool.tile([P, N2SUB], f32, tag="pmax")
                npairs = len(mm1_pairs)
                for j in range(N2SUB):
                    nmm = CCH * npairs
                    k = 0
                    for c in range(CCH):
                        for (t1, t2) in mm1_pairs:
                            nc.tensor.matmul(
                                st[:, j, :],
                                mm1_cast(t1[:, c, i * P:(i + 1) * P]),
                                mm1_cast(t2[:, c, j * 512:(j + 1) * 512]),
                                start=(k == 0),
                                stop=(k == nmm - 1),
                            )
                            k += 1
                    nc.vector.reduce_max(
                        pmax[:, j:j + 1], st[:, j, :], axis=mybir.AxisListType.X
                    )
                # ---- softmax over n2 (free axis) ----
                negmax = stat_pool.tile([P, 1], f32, tag="negmax")
                nc.vector.tensor_reduce(
                    negmax[:], pmax[:], axis=mybir.AxisListType.X,
                    op=mybir.AluOpType.max, negate=True,
                )
                psub = stat_pool.tile([P, N2SUB], f32, tag="psub")
                at = at_pool.tile([P, N2], f32, tag="at")
                for j in range(N2SUB):
                    nc.scalar.activation(
                        at[:, j * 512:(j + 1) * 512],
                        st[:, j, :],
                        mybir.ActivationFunctionType.Exp,
                        bias=negmax[:],
                        scale=1.0,
                        accum_out=psub[:, j:j + 1],
                    )
                sumexp = stat_pool.tile([P, 1], f32, tag="sumexp")
                nc.vector.reduce_sum(sumexp[:], psub[:], axis=mybir.AxisListType.X)
                rsum = stat_pool.tile([P, 1], f32, tag="rsum")
                nc.vector.reciprocal(rsum[:], sumexp[:])
                nc.vector.tensor_scalar_mul(at[:], at[:], rsum[:])

                # Software pipeline: emit previous chunk's transposes/MM2
                # after this chunk's MM1 so the PE never waits on softmax.
                if pending is not None:
                    emit_tail(*pending)
                pending = (at, re2t, b, i)
         emit_tail(*pending)

        if timing:
            with tc.For_i(0, reps, 1):
                emit_workload()
        else:
            assert reps == 1
            emit_workload()
        if dummy_d is not None:
            nc.sync.dma_start(dummy_d[:], ident[:, 0:1])

    nc.compile()
    return nc


def _get_nc(reps=1, timing=False):
    key = f"nc{reps}_{int(timing)}"
    if key not in _CACHE:
        _CACHE[key] = _build(reps, timing=timing)
    return _CACHE[key]


def _split_f16(x):
    hi = x.astype(np.float16)
    lo = (x - hi.astype(np.float32)).astype(np.float16)
    return hi, lo


def kernel(RI_1, RI_2, RE_2):
    from concourse.bass_utils import run_bass_kernel_spmd

    RI_1 = np.asarray(RI_1, dtype=np.float32)
    RI_2 = np.asarray(RI_2, dtype=np.float32)
    RE_2 = np.asarray(RE_2, dtype=np.float32)
    B = RI_1.shape[0]
    ncores = B // BP
    nc = _get_nc()

    RE_2T = np.ascontiguousarray(RE_2.transpose(0, 2, 1))  # [B, N2, D]

    in_maps = []
    for m in range(ncores):
        sl = slice(m * BP, (m + 1) * BP)
        im = {"re2t": RE_2T[sl]}
        if MM1_MODE == "f16x3":
            h1, l1 = _split_f16(RI_1[sl])
            h2, l2 = _split_f16(RI_2[sl])
            im.update(ri1_hi=h1, ri1_lo=l1, ri2_hi=h2, ri2_lo=l2)
        else:
            im.update(ri1=np.ascontiguousarray(RI_1[sl]),
                      ri2=np.ascontiguousarray(RI_2[sl]))
        in_maps.append(im)

    res = run_bass_kernel_spmd(nc, in_maps, list(range(ncores)))
    _CACHE["last"] = res
    _CACHE["in_maps"] = in_maps

    attention_map = np.concatenate([r["attn"] for r in res.results], axis=0)
    et = np.concatenate([r["et"] for r in res.results], axis=0)  # [B, N1, D]
    RE_embed = np.ascontiguousarray(et.transpose(0, 2, 1))       # [B, D, N1]
    return RE_embed, attention_map


# revision 18
# speedup vs baseline: 2.6566x; 2.6566x over previous
"""Trainium2 Bass kernel for nn_Aligner (B=16, C=D=512, N1=N2=2048).

  scores[b,n2,n1] = sum_c RI_2[b,c,n2] * RI_1[b,c,n1]
  attention_map   = softmax(scores, axis=1)          # over n2
  RE_embed[b,d,n1]= sum_n2 RE_2[b,d,n2] * attention_map[b,n2,n1]
  returns (RE_embed, attention_map)

Sharding: data-parallel over batch, 2 batches per core on 8 cores.

Per-core plan (per batch, per n1-chunk i of 128):
  MM1:  ST_i[n1,n2] = RI_1_chunk.T @ RI_2  (scores transposed, so softmax
        reduces along the free axis).  PSUM [128, 4x512] (4 banks).
  softmax: DVE reduce_max per 512-subtile -> combine -> ACT exp with
        bias=-max and accum_out partial sums -> DVE reciprocal ->
        DVE tensor_scalar mul (normalize).  AT_i [128, 2048] in SBUF.
  PE transposes: 16x [128,128] blocks AT_i -> A natural layout blocks
        (n2 partitions), via PSUM, ACT-copied to SBUF.
  MM2:  ET_i[n1, d] = sum_j A_block_j.T @ RE_2T_j, PSUM accumulate over
        16 n2-chunks.  Stored transposed; host transposes RE_embed back.
  DMA out: A natural blocks -> attention_map[b, :, i*128:(i+1)*128],
        ET_i -> et[b, i*128:(i+1)*128, :].
"""

import numpy as np

# Matmul precision modes: "f32" (exact, 4 cyc/row), "f32r" (1 cyc/row,
# reduced-precision fp32 streaming), "f16x3" (3x fp16 matmuls w/ hi+lo
# split, ~fp32 accuracy at 3 cyc/row).
MM1_MODE = "f32r"
MM2_MODE = "f32r"

P = 128
BP = 2            # batches per core
C = 512
CCH = C // P      # 4 contraction chunks
N1 = 2048
N1CH = N1 // P    # 16 n1 chunks
N2 = 2048
N2SUB = N2 // 512  # 4 n2 subtiles of 512 for MM1
NJ = N2 // P      # 16 n2 chunks of 128 for transposes / MM2
D = 512

_CACHE = {}


def _build(reps=1, timing=False):
    from contextlib import ExitStack

    import concourse.bacc as bacc
    import concourse.mybir as mybir
    import concourse.tile as tile
    from concourse.masks import make_identity

    dt = mybir.dt
    f32 = dt.float32
    f32r = dt.float32r
    f16 = dt.float16

    nc = bacc.Bacc("TRN2", target_bir_lowering=False, debug=False, num_devices=8)

    # DRAM I/O (per core shard: 2 batches).  In timing mode all large
    # tensors are Internal (device scratch, no host transfer) so wall
    # time ~= dispatch + HW exec; data is garbage but timing-equivalent.
    kin = "Internal" if timing else "ExternalInput"
    kout = "Internal" if timing else "ExternalOutput"
    mm1_dt = f32r if MM1_MODE == "f32r" else f32
    mm2_dt = f32r if MM2_MODE == "f32r" else f32
    if MM1_MODE == "f16x3":
        ri1_hi = nc.dram_tensor("ri1_hi", [BP, C, N1], f16, kind=kin).ap()
        ri1_lo = nc.dram_tensor("ri1_lo", [BP, C, N1], f16, kind=kin).ap()
        ri2_hi = nc.dram_tensor("ri2_hi", [BP, C, N2], f16, kind=kin).ap()
        ri2_lo = nc.dram_tensor("ri2_lo", [BP, C, N2], f16, kind=kin).ap()
    else:
        ri1_d = nc.dram_tensor("ri1", [BP, C, N1], mm1_dt, kind=kin).ap()
        ri2_d = nc.dram_tensor("ri2", [BP, C, N2], mm1_dt, kind=kin).ap()
    re2t_d = nc.dram_tensor("re2t", [BP, N2, D], mm2_dt, kind=kin).ap()
    attn_d = nc.dram_tensor("attn", [BP, N2, N1], f32, kind=kout).ap()
    et_d = nc.dram_tensor("et", [BP, N1, D], f32, kind=kout).ap()
    dummy_d = None
    if timing:
        dummy_d = nc.dram_tensor("tout", [P, 1], f32, kind="ExternalOutput").ap()

    with ExitStack() as ctx:
        tc = ctx.enter_context(tile.TileContext(nc))
        const = ctx.enter_context(tc.tile_pool(name="const", bufs=1))
        ident = const.tile([P, P], f32)
        make_identity(nc, ident)

        ri_pool = ctx.enter_context(tc.tile_pool(name="ri", bufs=2))
        re_pool = ctx.enter_context(tc.tile_pool(name="re", bufs=1))
        at_pool = ctx.enter_context(tc.tile_pool(name="at", bufs=2))
        anat_pool = ctx.enter_context(tc.tile_pool(name="anat", bufs=2))
        et_pool = ctx.enter_context(tc.tile_pool(name="etsb", bufs=2))
        stat_pool = ctx.enter_context(tc.tile_pool(name="stat", bufs=3))
        st_pool = ctx.enter_context(tc.tile_pool(name="st", bufs=1, space="PSUM"))
        tp_pool = ctx.enter_context(tc.tile_pool(name="tp", bufs=2, space="PSUM"))
        etp_pool = ctx.enter_context(tc.tile_pool(name="etp", bufs=2, space="PSUM"))

        def emit_tail(at, re2t, b, i):
            """Transposes + MM2 + output DMAs for chunk (b, i)."""
            anat = anat_pool.tile([P, NJ, P], mm2_dt, tag="anat")
            anat_flat = anat.rearrange("p j n -> p (j n)")
            for g in range(4):
                tp = tp_pool.tile([P, 4 * P], f32, tag="tp")
                for k in range(4):
                    jj = g * 4 + k
                    nc.tensor.transpose(
                        tp[:, k * P:(k + 1) * P],
                        at[:, jj * P:(jj + 1) * P],
                        ident,
                    )
                # ACT copy PSUM->SBUF; with mm2_dt=f32r this also performs
                # the rounding the BIR verifier demands of f32r producers.
                nc.scalar.copy(anat_flat[:, g * 512:(g + 1) * 512], tp[:])
            # attention_map natural-layout store
            nc.sync.dma_start(
                attn_d[b].rearrange("(j p) n -> p j n", p=P)[:, :, i * P:(i + 1) * P],
                anat.bitcast(f32),
            )
            # MM2: ET_i[n1,d] accumulate over n2 chunks
            etp = etp_pool.tile([P, D], f32, tag="etp")
            for jj in range(NJ):
                nc.tensor.matmul(
                    etp[:],
                    anat[:, jj, :],
                    re2t[:, jj, :],
                    start=(jj == 0),
                    stop=(jj == NJ - 1),
                )
            et_sb = et_pool.tile([P, D], f32, tag="et_sb")
            nc.vector.tensor_copy(et_sb[:], etp[:])
            nc.sync.dma_start(et_d[b, i * P:(i + 1) * P, :], et_sb[:])

        def emit_workload():
         pending = None
         for b in range(BP):
            if MM1_MODE == "f16x3":
                r1h = ri_pool.tile([P, CCH, N1], f16, tag="r1h")
                r1l = ri_pool.tile([P, CCH, N1], f16, tag="r1l")
                r2h = ri_pool.tile([P, CCH, N2], f16, tag="r2h")
                r2l = ri_pool.tile([P, CCH, N2], f16, tag="r2l")
                nc.sync.dma_start(r1h[:], ri1_hi[b].rearrange("(c p) n -> p c n", p=P))
                nc.sync.dma_start(r1l[:], ri1_lo[b].rearrange("(c p) n -> p c n", p=P))
                nc.sync.dma_start(r2h[:], ri2_hi[b].rearrange("(c p) n -> p c n", p=P))
                nc.sync.dma_start(r2l[:], ri2_lo[b].rearrange("(c p) n -> p c n", p=P))
                mm1_pairs = [(r1h, r2h), (r1h, r2l), (r1l, r2h)]
            else:
                ri1 = ri_pool.tile([P, CCH, N1], mm1_dt, tag="ri1")
                ri2 = ri_pool.tile([P, CCH, N2], mm1_dt, tag="ri2")
                nc.sync.dma_start(ri1[:], ri1_d[b].rearrange("(c p) n -> p c n", p=P))
                nc.sync.dma_start(ri2[:], ri2_d[b].rearrange("(c p) n -> p c n", p=P))
                mm1_pairs = [(ri1, ri2)]
            re2t = re_pool.tile([P, NJ, D], mm2_dt, tag="re2t")
            nc.sync.dma_start(re2t[:], re2t_d[b].rearrange("(j p) d -> p j d", p=P))

            for i in range(N1CH):
                # ---- MM1: ST_i [128 (n1), 2048 (n2)] in PSUM ----
                st = st_pool.tile([P, N2SUB, 512], f32, tag="st")
                pmax = stat_pool.tile([P, N2SUB], f32, tag="pmax")
                npairs = len(mm1_pairs)
                for j in range(N2SUB):
                    nmm = CCH * npairs
                    k = 0
                    for c in range(CCH):
                        for (t1, t2) in mm1_pairs:
                            nc.tensor.matmul(
                                st[:, j, :],
                                t1[:, c, i * P:(i + 1) * P],
                                t2[:, c, j * 512:(j + 1) * 512],
                                start=(k == 0),
                                stop=(k == nmm - 1),
                            )
                            k += 1
                    nc.vector.reduce_max(
                        pmax[:, j:j + 1], st[:, j, :], axis=mybir.AxisListType.X
                    )
                # ---- softmax over n2 (free axis) ----
                negmax = stat_pool.tile([P, 1], f32, tag="negmax")
                nc.vector.tensor_reduce(
                    negmax[:], pmax[:], axis=mybir.AxisListType.X,
                    op=mybir.AluOpType.max, negate=True,
                )
                psub = stat_pool.tile([P, N2SUB], f32, tag="psub")
                at = at_pool.tile([P, N2], f32, tag="at")
                for j in range(N2SUB):
                    nc.scalar.activation(
                        at[:, j * 512:(j + 1) * 512],
                        st[:, j, :],
                        mybir.ActivationFunctionType.Exp,
                        bias=negmax[:],
                        scale=1.0,
                        accum_out=psub[:, j:j + 1],
                    )
                sumexp = stat_pool.tile([P, 1], f32, tag="sumexp")
                nc.vector.reduce_sum(sumexp[:], psub[:], axis=mybir.AxisListType.X)
                rsum = stat_pool.tile([P, 1], f32, tag="rsum")
                nc.vector.reciprocal(rsum[:], sumexp[:])
                nc.vector.tensor_scalar_mul(at[:], at[:], rsum[:])

                # Software pipeline: emit previous chunk's transposes/MM2
                # after this chunk's MM1 so the PE never waits on softmax.
                if pending is not None:
                    emit_tail(*pending)
                pending = (at, re2t, b, i)
         emit_tail(*pending)

        if timing:
            with tc.For_i(0, reps, 1):
                emit_workload()
        else:
            assert reps == 1
            emit_workload()
        if dummy_d is not None:
            nc.sync.dma_start(dummy_d[:], ident[:, 0:1])

    nc.compile()
    return nc


def _get_nc(reps=1, timing=False):
    key = f"nc{reps}_{int(timing)}"
    if key not in _CACHE:
        _CACHE[key] = _build(reps, timing=timing)
    return _CACHE[key]


def _split_f16(x):
    hi = x.astype(np.float16)
    lo = (x - hi.astype(np.float32)).astype(np.float16)
    return hi, lo


def kernel(RI_1, RI_2, RE_2):
    from concourse.bass_utils import run_bass_kernel_spmd

    RI_1 = np.asarray(RI_1, dtype=np.float32)
    RI_2 = np.asarray(RI_2, dtype=np.float32)
    RE_2 = np.asarray(RE_2, dtype=np.float32)
    B = RI_1.shape[0]
    ncores = B // BP
    nc = _get_nc()

    RE_2T = np.ascontiguousarray(RE_2.transpose(0, 2, 1))  # [B, N2, D]

    in_maps = []
    for m in range(ncores):
        sl = slice(m * BP, (m + 1) * BP)
        im = {"re2t": RE_2T[sl]}
        if MM1_MODE == "f16x3":
            h1, l1 = _split_f16(RI_1[sl])
            h2, l2 = _split_f16(RI_2[sl])
            im.update(ri1_hi=h1, ri1_lo=l1, ri2_hi=h2, ri2_lo=l2)
        else:
            im.update(ri1=np.ascontiguousarray(RI_1[sl]),
                      ri2=np.ascontiguousarray(RI_2[sl]))
        in_maps.append(im)

    res = run_bass_kernel_spmd(nc, in_maps, list(range(ncores)))
    _CACHE["last"] = res
    _CACHE["in_maps"] = in_maps

    attention_map = np.concatenate([r["attn"] for r in res.results], axis=0)
    et = np.concatenate([r["et"] for r in res.results], axis=0)  # [B, N1, D]
    RE_embed = np.ascontiguousarray(et.transpose(0, 2, 1))       # [B, D, N1]
    return RE_embed, attention_map
